# revision 10
# baseline (speedup 1.0000x reference)
"""Trainium2 Bass kernel for gnn_message_passing (nn_MLP_43130061586864).

Strategy (8 NeuronCores, data-parallel over nodes, split at graph boundaries):
  - batch is sorted, so each graph (segment) is a contiguous node range.
  - Host pads each segment's node list to a multiple of F=512 and assigns
    cores contiguous blocks of B/8 = 512 segments. Every 512-node "row" then
    contains nodes of exactly one segment, so the gathered poi values become
    per-partition scalars on device (no per-node gather needed). Pad slots
    get t = poi_t[s], pos = poi_pos[s], which makes diff=0 and hence a
    contribution of exactly 0.
  - Device: tiles of [128 rows x 512 nodes]; feature prep + final scaling on
    DVE/ACT; the 2-10-20-10-5-1 MLP as block-diagonal-packed matmuls on the
    tensor engine (12 rows per matmul group, channels along partitions,
    nodes along the free dim). Matmul operands/outputs must sit at 32-aligned
    base partitions, so moving operands use the enclosing aligned window with
    zero rows in the (host-built) stationary.
  - Per-row sums via fused accumulators; row->segment reduction via a one-hot
    matmul accumulated in PSUM. Output: per-core partials [2, 512] -> concat.
"""

import numpy as np

import concourse.bass as bass
import concourse.tile as tile
from concourse import bacc, mybir
from concourse.bass_utils import run_bass_kernel_spmd

N = 8388608
B = 4096
NCORES = 8
SEGS = B // NCORES  # 512 segments per core
F = 512             # nodes per row == moving free dim == output segment count
P = 128             # rows per tile
FP32 = mybir.dt.float32
F32R = mybir.dt.float32r
EPS = 1e-12

USE_F32R = True
MMDT = F32R if USE_F32R else FP32

# group layout along the 128 rows of a tile: 10 groups of 12 + 1 group of 8
GROUPS = [(12 * i, 12) for i in range(10)] + [(120, 8)]
# enclosing 32-aligned window (start, size) for each group's row range
WIN = [(0, 32), (0, 32), (0, 64), (32, 32), (32, 32), (0, 128),
       (64, 32), (64, 32), (96, 32), (96, 32), (96, 32)]


def _mm_dt(ap):
    return ap.bitcast(F32R) if USE_F32R else ap


def build_nc(T, reps=1):
    """Build the SPMD program for T tiles (R = T*128 rows) per core.

    reps > 1 repeats the whole tile loop (for timing-slope measurement);
    the output is overwritten each rep, so results are unchanged."""
    nc = bacc.Bacc(None, target_bir_lowering=False, debug=False)
    R = T * P

    # ---- DRAM parameters (inputs) ----
    d_t = nc.declare_dram_parameter("tt", [R, F], FP32, isOutput=False)
    d_px = nc.declare_dram_parameter("px", [R, F], FP32, isOutput=False)
    d_py = nc.declare_dram_parameter("py", [R, F], FP32, isOutput=False)
    d_rm = nc.declare_dram_parameter("rmeta", [R, 4], FP32, isOutput=False)
    d_s1d = nc.declare_dram_parameter("s1d", [128, 120 * 11], MMDT, isOutput=False)
    d_s1r = nc.declare_dram_parameter("s1r", [128, 120 * 11], MMDT, isOutput=False)
    d_s2 = nc.declare_dram_parameter("s2", [60, 120], MMDT, isOutput=False)
    d_s2b = nc.declare_dram_parameter("s2b", [120, 120], MMDT, isOutput=False)
    d_s3a = nc.declare_dram_parameter("s3a", [120, 124], MMDT, isOutput=False)
    d_s3b = nc.declare_dram_parameter("s3b", [120, 124], MMDT, isOutput=False)
    d_s4 = nc.declare_dram_parameter("s4", [124, 60], MMDT, isOutput=False)
    d_s5 = nc.declare_dram_parameter("s5", [60, 128 * 11], MMDT, isOutput=False)
    d_bias = nc.declare_dram_parameter("bias", [128, 5], FP32, isOutput=False)
    d_out = nc.declare_dram_parameter("part", [2, F], FP32, isOutput=True)

    with tile.TileContext(nc) as tc:
        with (
            tc.tile_pool(name="consts", bufs=1) as cpool,
            tc.tile_pool(name="inp", bufs=2) as ipool,
            tc.tile_pool(name="work", bufs=2) as wpool,
            tc.tile_pool(name="hact", bufs=2) as hpool,
            tc.tile_pool(name="pz1", bufs=2, space="PSUM") as pz1,
            tc.tile_pool(name="pz2", bufs=2, space="PSUM") as pz2,
            tc.tile_pool(name="pz3", bufs=1, space="PSUM") as pz3,
            tc.tile_pool(name="pz4", bufs=1, space="PSUM") as pz4,
            tc.tile_pool(name="pw", bufs=1, space="PSUM") as pwp,
            tc.tile_pool(name="pacc", bufs=1, space="PSUM") as paccp,
        ):
            # ---- constants ----
            s1d = cpool.tile([128, 120 * 11], MMDT)
            s1r = cpool.tile([128, 120 * 11], MMDT)
            s2 = cpool.tile([60, 120], MMDT)
            s2b = cpool.tile([120, 120], MMDT)
            s3a = cpool.tile([120, 124], MMDT)
            s3b = cpool.tile([120, 124], MMDT)
            s4 = cpool.tile([124, 60], MMDT)
            s5 = cpool.tile([60, 128 * 11], MMDT)
            bias = cpool.tile([128, 5], FP32)
            nc.sync.dma_start(out=s1d[:], in_=d_s1d[:])
            nc.sync.dma_start(out=s1r[:], in_=d_s1r[:])
            nc.sync.dma_start(out=s2[:], in_=d_s2[:])
            nc.sync.dma_start(out=s2b[:], in_=d_s2b[:])
            nc.sync.dma_start(out=s3a[:], in_=d_s3a[:])
            nc.sync.dma_start(out=s3b[:], in_=d_s3b[:])
            nc.sync.dma_start(out=s4[:], in_=d_s4[:])
            nc.sync.dma_start(out=s5[:], in_=d_s5[:])
            nc.sync.dma_start(out=bias[:], in_=d_bias[:])

            iota_i = cpool.tile([P, F], mybir.dt.int32)
            iota_f = cpool.tile([P, F], FP32)
            nc.gpsimd.iota(iota_i[:], [[1, F]], channel_multiplier=0)
            nc.vector.tensor_copy(out=iota_f[:], in_=iota_i[:])

            acc = paccp.tile([2, F], FP32)

            for rep in range(reps):
              for tau in range(T):
                r0 = tau * P
                t_t = ipool.tile([P, F], FP32, tag="t_t")
                px_t = ipool.tile([P, F], FP32, tag="px_t")
                py_t = ipool.tile([P, F], FP32, tag="py_t")
                rm = ipool.tile([P, 4], FP32, tag="rm")
                nc.sync.dma_start(out=t_t[:], in_=d_t[r0:r0 + P, :])
                nc.sync.dma_start(out=px_t[:], in_=d_px[r0:r0 + P, :])
                nc.sync.dma_start(out=py_t[:], in_=d_py[r0:r0 + P, :])
                nc.sync.dma_start(out=rm[:], in_=d_rm[r0:r0 + P, :])

                # ---- feature prep ----
                fd = wpool.tile([P, F], MMDT, tag="fd")     # t - poi_t
                dx = wpool.tile([P, F], FP32, tag="dx")
                dy = wpool.tile([P, F], FP32, tag="dy")
                dx2 = wpool.tile([P, F], FP32, tag="dx2")
                dy2 = wpool.tile([P, F], FP32, tag="dy2")
                r2 = wpool.tile([P, F], MMDT, tag="r2")
                # rmeta columns: 0=-poi_t, 1=-poi_x, 2=-poi_y, 3=seg_local
                nc.scalar.activation(fd[:], t_t[:],
                                     mybir.ActivationFunctionType.Identity,
                                     bias=rm[:, 0:1])
                nc.vector.tensor_scalar(out=dx[:], in0=px_t[:],
                                        scalar1=rm[:, 1:2], scalar2=None,
                                        op0=mybir.AluOpType.add)
                nc.vector.tensor_scalar(out=dy[:], in0=py_t[:],
                                        scalar1=rm[:, 2:3], scalar2=None,
                                        op0=mybir.AluOpType.add)
                nc.vector.tensor_tensor(out=dx2[:], in0=dx[:], in1=dx[:],
                                        op=mybir.AluOpType.mult)
                nc.scalar.activation(dy2[:], dy[:],
                                     mybir.ActivationFunctionType.Square)
                nc.vector.tensor_tensor(out=r2[:], in0=dx2[:], in1=dy2[:],
                                        op=mybir.AluOpType.add)

                # norm path: inv = 1 / max(sqrt(r2), EPS)
                m_t = wpool.tile([P, F], FP32, tag="m_t")
                nrm = wpool.tile([P, F], FP32, tag="nrm")
                inv = wpool.tile([P, F], FP32, tag="inv")
                nc.vector.tensor_scalar(out=m_t[:], in0=r2[:].bitcast(FP32),
                                        scalar1=float(EPS * EPS), scalar2=None,
                                        op0=mybir.AluOpType.max)
                nc.scalar.activation(nrm[:], m_t[:],
                                     mybir.ActivationFunctionType.Sqrt)
                nc.vector.reciprocal(out=inv[:], in_=nrm[:])

                # ---- MLP: w for all 128 rows of the tile ----
                wbank = pwp.tile([P, F], FP32, tag="wbank")
                z3 = pz3.tile([124, F], FP32, tag="z3")
                for j, (g0, gs) in enumerate(GROUPS):
                    g = j
                    w0, kw = WIN[g]
                    h6 = min(6, gs)          # chunks in the first half
                    hr = gs - h6             # chunks in the second half
                    z1 = pz1.tile([120, F], FP32, tag="z1")
                    nc.tensor.matmul(z1[:10 * gs, :],
                                     _mm_dt(s1d[w0:w0 + kw,
                                                120 * g:120 * g + 10 * gs]),
                                     _mm_dt(fd[w0:w0 + kw, :]),
                                     start=True, stop=False,
                                     tile_position=(w0, 0))
                    nc.tensor.matmul(z1[:10 * gs, :],
                                     _mm_dt(s1r[w0:w0 + kw,
                                                120 * g:120 * g + 10 * gs]),
                                     _mm_dt(r2[w0:w0 + kw, :]),
                                     start=False, stop=True,
                                     tile_position=(w0, 0))
                    h1 = hpool.tile([120, F], MMDT, tag="h1")
                    nc.scalar.activation(h1[:10 * gs, :], z1[:10 * gs, :],
                                         mybir.ActivationFunctionType.Relu,
                                         bias=bias[:10 * gs, 0:1])

                    z2a = pz2.tile([120, F], FP32, tag="z2")
                    nc.tensor.matmul(z2a[:20 * h6, :],
                                     _mm_dt(s2[:10 * h6, :20 * h6]),
                                     _mm_dt(h1[0:10 * h6, :]),
                                     start=True, stop=True,
                                     tile_position=(0, 0))
                    h2a = hpool.tile([120, F], MMDT, tag="h2a")
                    nc.scalar.activation(h2a[:20 * h6, :], z2a[:20 * h6, :],
                                         mybir.ActivationFunctionType.Relu,
                                         bias=bias[:20 * h6, 1:2])
                    z2b = pz2.tile([120, F], FP32, tag="z2")
                    nc.tensor.matmul(z2b[:20 * hr, :],
                                     _mm_dt(s2b[:10 * gs, :20 * hr]),
                                     _mm_dt(h1[0:10 * gs, :]),
                                     start=True, stop=True,
                                     tile_position=(0, 0))
                    h2b = hpool.tile([120, F], MMDT, tag="h2b")
                    nc.vector.tensor_scalar(out=h2b[:20 * hr, :],
                                            in0=z2b[:20 * hr, :],
                                            scalar1=bias[:20 * hr, 1:2],
                                            scalar2=0.0,
                                            op0=mybir.AluOpType.add,
                                            op1=mybir.AluOpType.max)

                    nc.tensor.matmul(z3[0:124, :],
                                     _mm_dt(s3a[:20 * h6, :124]),
                                     _mm_dt(h2a[:20 * h6, :]),
                                     start=True, stop=False,
                                     tile_position=(0, 0))
                    nc.tensor.matmul(z3[0:124, :],
                                     _mm_dt(s3b[:20 * hr, :124]),
                                     _mm_dt(h2b[:20 * hr, :]),
                                     start=False, stop=True,
                                     tile_position=(0, 0))
                    h3 = hpool.tile([124, F], MMDT, tag="h3")
                    nh3 = 64 + 10 * hr
                    nc.vector.tensor_scalar(out=h3[:nh3, :],
                                            in0=z3[:nh3, :],
                                            scalar1=bias[:nh3, 2:3],
                                            scalar2=0.0,
                                            op0=mybir.AluOpType.add,
                                            op1=mybir.AluOpType.max)

                    z4 = pz4.tile([60, F], FP32, tag="z4")
                    nc.tensor.matmul(z4[:5 * gs, :],
                                     _mm_dt(s4[:nh3, :5 * gs]),
                                     _mm_dt(h3[:nh3, :]),
                                     start=True, stop=True,
                                     tile_position=(0, 0))
                    h4 = hpool.tile([60, F], MMDT, tag="h4")
                    nc.scalar.activation(h4[:5 * gs, :], z4[:5 * gs, :],
                                         mybir.ActivationFunctionType.Relu,
                                         bias=bias[:5 * gs, 3:4])

                    # w rows land in wbank via a full-width M window with
                    # zero columns outside this group's rows; the 11 matmuls
                    # form one accumulation group over the tile.
                    nc.tensor.matmul(wbank[0:P, :],
                                     _mm_dt(s5[:5 * gs, 128 * g:128 * (g + 1)]),
                                     _mm_dt(h4[:5 * gs, :]),
                                     start=(g == 0), stop=(g == len(GROUPS) - 1),
                                     tile_position=(0, 0),
                                     skip_group_check=True)

                # ---- contrib + row sums ----
                t1 = wpool.tile([P, F], FP32, tag="t1")
                cxs = wpool.tile([P, F], FP32, tag="cxs")
                cys = wpool.tile([P, F], FP32, tag="cys")
                rs2 = wpool.tile([P, 2], FP32, tag="rs2")
                onehot = wpool.tile([P, F], FP32, tag="onehot")
                # t1 = (w + b5) * inv
                nc.vector.scalar_tensor_tensor(out=t1[:], in0=wbank[:],
                                               scalar=bias[:, 4:5],
                                               in1=inv[:],
                                               op0=mybir.AluOpType.add,
                                               op1=mybir.AluOpType.mult)
                nc.vector.scalar_tensor_tensor(out=cxs[:], in0=t1[:],
                                               scalar=1.0, in1=dx[:],
                                               op0=mybir.AluOpType.mult,
                                               op1=mybir.AluOpType.mult,
                                               accum_out=rs2[:, 0:1])
                nc.vector.scalar_tensor_tensor(out=cys[:], in0=t1[:],
                                               scalar=1.0, in1=dy[:],
                                               op0=mybir.AluOpType.mult,
                                               op1=mybir.AluOpType.mult,
                                               accum_out=rs2[:, 1:2])
                # one-hot row->segment, accumulate into acc
                nc.vector.tensor_scalar(out=onehot[:], in0=iota_f[:],
                                        scalar1=rm[:, 3:4], scalar2=None,
                                        op0=mybir.AluOpType.is_equal)
                nc.tensor.matmul(acc[:], rs2[:], onehot[:],
                                 start=(tau == 0), stop=(tau == T - 1),
                                 skip_group_check=True)

            acc_sb = cpool.tile([2, F], FP32)
            nc.vector.tensor_copy(out=acc_sb[:], in_=acc[:])
            nc.sync.dma_start(out=d_out[:], in_=acc_sb[:])

    nc.compile()
    return nc


def _host_prep(t, pos, poi_t, poi_pos, batch):
    """Shard + pad at graph boundaries. Returns per-core input dicts and T."""
    t = np.ascontiguousarray(np.asarray(t, dtype=np.float32))
    pos = np.ascontiguousarray(np.asarray(pos, dtype=np.float32))
    poi_t = np.asarray(poi_t, dtype=np.float32)
    poi_pos = np.asarray(poi_pos, dtype=np.float32)
    batch = np.asarray(batch)

    bounds = np.searchsorted(batch, np.arange(B + 1)).astype(np.int64)
    counts = np.diff(bounds)                       # [B]
    rows_per_seg = -(-counts // F)                 # ceil, 0 for empty segs

    core_rows = [int(rows_per_seg[k * SEGS:(k + 1) * SEGS].sum())
                 for k in range(NCORES)]
    R_needed = max(core_rows)
    T = -(-R_needed // P)
    R = T * P

    per_core = []
    for k in range(NCORES):
        s0, s1 = k * SEGS, (k + 1) * SEGS
        rs = rows_per_seg[s0:s1]
        nrows = int(rs.sum())
        seg_of_row = np.repeat(np.arange(s0, s1), rs)          # [nrows]
        row_in_seg = (np.arange(nrows)
                      - np.repeat(np.cumsum(rs) - rs, rs))     # 0,1,.. per seg
        row_node0 = bounds[seg_of_row] + row_in_seg * F

        pad = R - nrows
        seg_of_row = np.concatenate(
            [seg_of_row, np.full(pad, s1 - 1, np.int64)])
        row_node0 = np.concatenate([row_node0, np.full(pad, -1, np.int64)])

        nidx = row_node0[:, None] + np.arange(F)[None, :]       # [R, F]
        row_end = bounds[seg_of_row + 1]
        valid = (row_node0[:, None] >= 0) & (nidx < row_end[:, None])
        nidx_c = np.where(valid, nidx, 0)

        seg_pt = poi_t[seg_of_row]
        seg_px = poi_pos[seg_of_row, 0]
        seg_py = poi_pos[seg_of_row, 1]

        tt = np.where(valid, t[nidx_c], seg_pt[:, None]).astype(np.float32)
        px = np.where(valid, pos[nidx_c, 0], seg_px[:, None]).astype(np.float32)
        py = np.where(valid, pos[nidx_c, 1], seg_py[:, None]).astype(np.float32)
        rmeta = np.stack([-seg_pt, -seg_px, -seg_py,
                          (seg_of_row - s0).astype(np.float32)],
                         axis=1).astype(np.float32)
        per_core.append({"tt": tt, "px": px, "py": py, "rmeta": rmeta})
    return per_core, T


def _stationaries(W1, b1, W2, b2, W3, b3, W4, b4, W5, b5):
    W1, W2, W3, W4, W5 = [np.asarray(w, np.float32) for w in (W1, W2, W3, W4, W5)]
    b1, b2, b3, b4, b5 = [np.asarray(b, np.float32) for b in (b1, b2, b3, b4, b5)]
    s1d = np.zeros((128, 120 * 11), np.float32)
    s1r = np.zeros((128, 120 * 11), np.float32)
    for g, (g0, gs) in enumerate(GROUPS):
        for c in range(gs):
            s1d[g0 + c, 120 * g + 10 * c:120 * g + 10 * c + 10] = W1[:, 0]
            s1r[g0 + c, 120 * g + 10 * c:120 * g + 10 * c + 10] = W1[:, 1]
    s2 = np.zeros((60, 120), np.float32)
    for c in range(6):
        s2[10 * c:10 * c + 10, 20 * c:20 * c + 20] = W2.T
    s2b = np.zeros((120, 120), np.float32)
    s2b[60:120, :] = s2
    s3a = np.zeros((120, 124), np.float32)
    s3b = np.zeros((120, 124), np.float32)
    for c in range(6):
        s3a[20 * c:20 * c + 20, 10 * c:10 * c + 10] = W3.T
        s3b[20 * c:20 * c + 20, 64 + 10 * c:64 + 10 * c + 10] = W3.T
    s4 = np.zeros((124, 60), np.float32)
    for c in range(6):
        s4[10 * c:10 * c + 10, 5 * c:5 * c + 5] = W4.T
    for c in range(6):
        s4[64 + 10 * c:64 + 10 * c + 10, 5 * (6 + c):5 * (6 + c) + 5] = W4.T
    s5 = np.zeros((60, 128 * 11), np.float32)
    for g, (g0, gs) in enumerate(GROUPS):
        for c in range(gs):
            s5[5 * c:5 * c + 5, 128 * g + g0 + c] = W5[0]
    bias = np.zeros((128, 5), np.float32)
    bias[:120, 0] = np.tile(b1, 12)
    bias[:120, 1] = np.tile(b2, 6)
    bias[:60, 2] = np.tile(b3, 6)
    bias[64:124, 2] = np.tile(b3, 6)
    bias[:60, 3] = np.tile(b4, 12)
    bias[:, 4] = b5[0]
    return {"s1d": s1d, "s1r": s1r, "s2": s2, "s2b": s2b, "s3a": s3a,
            "s3b": s3b, "s4": s4, "s5": s5, "bias": bias}


_NC_CACHE = {}


def kernel(t, pos, poi_t, poi_pos, batch,
           W1, b1, W2, b2, W3, b3, W4, b4, W5, b5):
    per_core, T = _host_prep(t, pos, poi_t, poi_pos, batch)
    sta = _stationaries(W1, b1, W2, b2, W3, b3, W4, b4, W5, b5)

    if T not in _NC_CACHE:
        _NC_CACHE[T] = build_nc(T)
    nc = _NC_CACHE[T]

    in_maps = [{**core_inputs, **sta} for core_inputs in per_core]
    res = run_bass_kernel_spmd(nc, in_maps, list(range(NCORES)))
    global LAST_RESULT
    LAST_RESULT = res

    out = np.zeros((B, 2), np.float32)
    for k in range(NCORES):
        part = res.results[k]["part"]          # [2, 512]
        out[k * SEGS:(k + 1) * SEGS, :] = part.T
    return out


# revision 11
# speedup vs baseline: 1.3306x; 1.3306x over previous
"""Trainium2 Bass kernel for gnn_message_passing (nn_MLP_43130061586864).

Strategy (8 NeuronCores, data-parallel over nodes, split at graph boundaries):
  - batch is sorted, so each graph (segment) is a contiguous node range.
  - Host pads each segment's node list to a multiple of F=512 and assigns
    cores contiguous blocks of B/8 = 512 segments. Every 512-node "row" then
    contains nodes of exactly one segment, so the gathered poi values become
    per-partition scalars on device (no per-node gather needed). Pad slots
    get t = poi_t[s], pos = poi_pos[s], which makes diff=0 and hence a
    contribution of exactly 0.
  - Device: tiles of [128 rows x 512 nodes]; feature prep + final scaling on
    DVE/ACT; the 2-10-20-10-5-1 MLP as block-diagonal-packed matmuls on the
    tensor engine (12 rows per matmul group, channels along partitions,
    nodes along the free dim). Matmul operands/outputs must sit at 32-aligned
    base partitions, so moving operands use the enclosing aligned window with
    zero rows in the (host-built) stationary.
  - Per-row sums via fused accumulators; row->segment reduction via a one-hot
    matmul accumulated in PSUM. Output: per-core partials [2, 512] -> concat.
"""

import numpy as np

import concourse.bass as bass
import concourse.tile as tile
from concourse import bacc, mybir
from concourse.bass_utils import run_bass_kernel_spmd

N = 8388608
B = 4096
NCORES = 8
SEGS = B // NCORES  # 512 segments per core
F = 512             # nodes per row == moving free dim == output segment count
P = 128             # rows per tile
FP32 = mybir.dt.float32
F32R = mybir.dt.float32r
EPS = 1e-12

USE_F32R = True
MMDT = F32R if USE_F32R else FP32

# group layout along the 128 rows of a tile: 10 groups of 12 + 1 group of 8
GROUPS = [(12 * i, 12) for i in range(10)] + [(120, 8)]
# enclosing 32-aligned window (start, size) for each group's row range
WIN = [(0, 32), (0, 32), (0, 64), (32, 32), (32, 32), (0, 128),
       (64, 32), (64, 32), (96, 32), (96, 32), (96, 32)]


def _mm_dt(ap):
    return ap.bitcast(F32R) if USE_F32R else ap


def build_nc(T, reps=1, parts="full"):
    """Build the SPMD program for T tiles (R = T*128 rows) per core.

    reps > 1 repeats the whole tile loop (for timing-slope measurement);
    the output is overwritten each rep, so results are unchanged.
    parts: "full" | "nomlp" (skip matmul groups) | "nofinal" (skip norm+
    contrib math) — ablation variants for timing only."""
    nc = bacc.Bacc(None, target_bir_lowering=False, debug=False)
    R = T * P

    # ---- DRAM parameters (inputs) ----
    d_t = nc.declare_dram_parameter("tt", [R, F], FP32, isOutput=False)
    d_px = nc.declare_dram_parameter("px", [R, F], FP32, isOutput=False)
    d_py = nc.declare_dram_parameter("py", [R, F], FP32, isOutput=False)
    d_rm = nc.declare_dram_parameter("rmeta", [R, 4], FP32, isOutput=False)
    d_s1d = nc.declare_dram_parameter("s1d", [128, 120 * 11], MMDT, isOutput=False)
    d_s1r = nc.declare_dram_parameter("s1r", [128, 120 * 11], MMDT, isOutput=False)
    d_s2 = nc.declare_dram_parameter("s2", [60, 120], MMDT, isOutput=False)
    d_s2b = nc.declare_dram_parameter("s2b", [120, 120], MMDT, isOutput=False)
    d_s3a = nc.declare_dram_parameter("s3a", [120, 124], MMDT, isOutput=False)
    d_s3b = nc.declare_dram_parameter("s3b", [120, 124], MMDT, isOutput=False)
    d_s4 = nc.declare_dram_parameter("s4", [124, 60], MMDT, isOutput=False)
    d_s5 = nc.declare_dram_parameter("s5", [60, 128 * 11], MMDT, isOutput=False)
    d_bias = nc.declare_dram_parameter("bias", [128, 5], FP32, isOutput=False)
    d_out = nc.declare_dram_parameter("part", [2, F], FP32, isOutput=True)

    with tile.TileContext(nc) as tc:
        with (
            tc.tile_pool(name="consts", bufs=1) as cpool,
            tc.tile_pool(name="inp", bufs=2) as ipool,
            tc.tile_pool(name="work", bufs=2) as wpool,
            tc.tile_pool(name="hact", bufs=2) as hpool,
            tc.tile_pool(name="pz1", bufs=2, space="PSUM") as pz1,
            tc.tile_pool(name="pz2", bufs=2, space="PSUM") as pz2,
            tc.tile_pool(name="pz3", bufs=1, space="PSUM") as pz3,
            tc.tile_pool(name="pz4", bufs=1, space="PSUM") as pz4,
            tc.tile_pool(name="pw", bufs=1, space="PSUM") as pwp,
            tc.tile_pool(name="pacc", bufs=1, space="PSUM") as paccp,
        ):
            # ---- constants ----
            s1d = cpool.tile([128, 120 * 11], MMDT)
            s1r = cpool.tile([128, 120 * 11], MMDT)
            s2 = cpool.tile([60, 120], MMDT)
            s2b = cpool.tile([120, 120], MMDT)
            s3a = cpool.tile([120, 124], MMDT)
            s3b = cpool.tile([120, 124], MMDT)
            s4 = cpool.tile([124, 60], MMDT)
            s5 = cpool.tile([60, 128 * 11], MMDT)
            bias = cpool.tile([128, 5], FP32)
            nc.sync.dma_start(out=s1d[:], in_=d_s1d[:])
            nc.sync.dma_start(out=s1r[:], in_=d_s1r[:])
            nc.sync.dma_start(out=s2[:], in_=d_s2[:])
            nc.sync.dma_start(out=s2b[:], in_=d_s2b[:])
            nc.sync.dma_start(out=s3a[:], in_=d_s3a[:])
            nc.sync.dma_start(out=s3b[:], in_=d_s3b[:])
            nc.sync.dma_start(out=s4[:], in_=d_s4[:])
            nc.sync.dma_start(out=s5[:], in_=d_s5[:])
            nc.sync.dma_start(out=bias[:], in_=d_bias[:])

            iota_i = cpool.tile([P, F], mybir.dt.int32)
            iota_f = cpool.tile([P, F], FP32)
            nc.gpsimd.iota(iota_i[:], [[1, F]], channel_multiplier=0)
            nc.vector.tensor_copy(out=iota_f[:], in_=iota_i[:])

            acc = paccp.tile([2, F], FP32)

            for rep in range(reps):
              for tau in range(T):
                r0 = tau * P
                t_t = ipool.tile([P, F], FP32, tag="t_t")
                px_t = ipool.tile([P, F], FP32, tag="px_t")
                py_t = ipool.tile([P, F], FP32, tag="py_t")
                rm = ipool.tile([P, 4], FP32, tag="rm")
                nc.sync.dma_start(out=t_t[:], in_=d_t[r0:r0 + P, :])
                nc.sync.dma_start(out=px_t[:], in_=d_px[r0:r0 + P, :])
                nc.sync.dma_start(out=py_t[:], in_=d_py[r0:r0 + P, :])
                nc.sync.dma_start(out=rm[:], in_=d_rm[r0:r0 + P, :])

                # ---- feature prep ----
                fd = wpool.tile([P, F], MMDT, tag="fd")     # t - poi_t
                dx = wpool.tile([P, F], FP32, tag="dx")
                dy = wpool.tile([P, F], FP32, tag="dy")
                dx2 = wpool.tile([P, F], FP32, tag="dx2")
                dy2 = wpool.tile([P, F], FP32, tag="dy2")
                r2 = wpool.tile([P, F], MMDT, tag="r2")
                # rmeta columns: 0=-poi_t, 1=-poi_x, 2=-poi_y, 3=seg_local
                nc.scalar.activation(fd[:], t_t[:],
                                     mybir.ActivationFunctionType.Identity,
                                     bias=rm[:, 0:1])
                nc.vector.tensor_scalar(out=dx[:], in0=px_t[:],
                                        scalar1=rm[:, 1:2], scalar2=None,
                                        op0=mybir.AluOpType.add)
                nc.vector.tensor_scalar(out=dy[:], in0=py_t[:],
                                        scalar1=rm[:, 2:3], scalar2=None,
                                        op0=mybir.AluOpType.add)
                nc.vector.tensor_tensor(out=dx2[:], in0=dx[:], in1=dx[:],
                                        op=mybir.AluOpType.mult)
                nc.scalar.activation(dy2[:], dy[:],
                                     mybir.ActivationFunctionType.Square)
                nc.vector.tensor_tensor(out=r2[:], in0=dx2[:], in1=dy2[:],
                                        op=mybir.AluOpType.add)

                # norm path: inv = 1 / max(sqrt(r2), EPS)
                do_final = parts != "nofinal"
                m_t = wpool.tile([P, F], FP32, tag="m_t")
                nrm = wpool.tile([P, F], FP32, tag="nrm")
                inv = wpool.tile([P, F], FP32, tag="inv")
                if do_final:
                    nc.vector.tensor_scalar(out=m_t[:], in0=r2[:].bitcast(FP32),
                                            scalar1=float(EPS * EPS),
                                            scalar2=None,
                                            op0=mybir.AluOpType.max)
                    nc.scalar.activation(nrm[:], m_t[:],
                                         mybir.ActivationFunctionType.Sqrt)
                    nc.vector.reciprocal(out=inv[:], in_=nrm[:])

                # ---- MLP: w for all 128 rows of the tile ----
                wbank = pwp.tile([P, F], FP32, tag="wbank")
                z3 = pz3.tile([124, F], FP32, tag="z3")
                groups_iter = GROUPS if parts != "nomlp" else []
                if parts == "nomlp":
                    nc.vector.memset(wbank[:], 0.0)
                for j, (g0, gs) in enumerate(groups_iter):
                    g = j
                    w0, kw = WIN[g]
                    h6 = min(6, gs)          # chunks in the first half
                    hr = gs - h6             # chunks in the second half
                    z1 = pz1.tile([120, F], FP32, tag="z1")
                    nc.tensor.matmul(z1[:10 * gs, :],
                                     _mm_dt(s1d[w0:w0 + kw,
                                                120 * g:120 * g + 10 * gs]),
                                     _mm_dt(fd[w0:w0 + kw, :]),
                                     start=True, stop=False,
                                     tile_position=(w0, 0))
                    nc.tensor.matmul(z1[:10 * gs, :],
                                     _mm_dt(s1r[w0:w0 + kw,
                                                120 * g:120 * g + 10 * gs]),
                                     _mm_dt(r2[w0:w0 + kw, :]),
                                     start=False, stop=True,
                                     tile_position=(w0, 0))
                    h1 = hpool.tile([120, F], MMDT, tag="h1")
                    nc.scalar.activation(h1[:10 * gs, :], z1[:10 * gs, :],
                                         mybir.ActivationFunctionType.Relu,
                                         bias=bias[:10 * gs, 0:1])

                    z2a = pz2.tile([120, F], FP32, tag="z2")
                    nc.tensor.matmul(z2a[:20 * h6, :],
                                     _mm_dt(s2[:10 * h6, :20 * h6]),
                                     _mm_dt(h1[0:10 * h6, :]),
                                     start=True, stop=True,
                                     tile_position=(0, 0))
                    h2a = hpool.tile([120, F], MMDT, tag="h2a")
                    nc.scalar.activation(h2a[:20 * h6, :], z2a[:20 * h6, :],
                                         mybir.ActivationFunctionType.Relu,
                                         bias=bias[:20 * h6, 1:2])
                    z2b = pz2.tile([120, F], FP32, tag="z2")
                    nc.tensor.matmul(z2b[:20 * hr, :],
                                     _mm_dt(s2b[:10 * gs, :20 * hr]),
                                     _mm_dt(h1[0:10 * gs, :]),
                                     start=True, stop=True,
                                     tile_position=(0, 0))
                    h2b = hpool.tile([120, F], MMDT, tag="h2b")
                    nc.vector.tensor_scalar(out=h2b[:20 * hr, :],
                                            in0=z2b[:20 * hr, :],
                                            scalar1=bias[:20 * hr, 1:2],
                                            scalar2=0.0,
                                            op0=mybir.AluOpType.add,
                                            op1=mybir.AluOpType.max)

                    nc.tensor.matmul(z3[0:124, :],
                                     _mm_dt(s3a[:20 * h6, :124]),
                                     _mm_dt(h2a[:20 * h6, :]),
                                     start=True, stop=False,
                                     tile_position=(0, 0))
                    nc.tensor.matmul(z3[0:124, :],
                                     _mm_dt(s3b[:20 * hr, :124]),
                                     _mm_dt(h2b[:20 * hr, :]),
                                     start=False, stop=True,
                                     tile_position=(0, 0))
                    h3 = hpool.tile([124, F], MMDT, tag="h3")
                    nh3 = 64 + 10 * hr
                    nc.vector.tensor_scalar(out=h3[:nh3, :],
                                            in0=z3[:nh3, :],
                                            scalar1=bias[:nh3, 2:3],
                                            scalar2=0.0,
                                            op0=mybir.AluOpType.add,
                                            op1=mybir.AluOpType.max)

                    z4 = pz4.tile([60, F], FP32, tag="z4")
                    nc.tensor.matmul(z4[:5 * gs, :],
                                     _mm_dt(s4[:nh3, :5 * gs]),
                                     _mm_dt(h3[:nh3, :]),
                                     start=True, stop=True,
                                     tile_position=(0, 0))
                    h4 = hpool.tile([60, F], MMDT, tag="h4")
                    nc.scalar.activation(h4[:5 * gs, :], z4[:5 * gs, :],
                                         mybir.ActivationFunctionType.Relu,
                                         bias=bias[:5 * gs, 3:4])

                    # w rows land in wbank via a full-width M window with
                    # zero columns outside this group's rows; the 11 matmuls
                    # form one accumulation group over the tile.
                    nc.tensor.matmul(wbank[0:P, :],
                                     _mm_dt(s5[:5 * gs, 128 * g:128 * (g + 1)]),
                                     _mm_dt(h4[:5 * gs, :]),
                                     start=(g == 0), stop=(g == len(GROUPS) - 1),
                                     tile_position=(0, 0),
                                     skip_group_check=True)

                # ---- contrib + row sums ----
                t1 = wpool.tile([P, F], FP32, tag="t1")
                cxs = wpool.tile([P, F], FP32, tag="cxs")
                cys = wpool.tile([P, F], FP32, tag="cys")
                rs2 = wpool.tile([P, 2], FP32, tag="rs2")
                onehot = wpool.tile([P, F], FP32, tag="onehot")
                # t1 = (w + b5) * inv
                if do_final:
                    nc.vector.scalar_tensor_tensor(out=t1[:], in0=wbank[:],
                                                   scalar=bias[:, 4:5],
                                                   in1=inv[:],
                                                   op0=mybir.AluOpType.add,
                                                   op1=mybir.AluOpType.mult)
                    nc.vector.scalar_tensor_tensor(out=cxs[:], in0=t1[:],
                                                   scalar=1.0, in1=dx[:],
                                                   op0=mybir.AluOpType.mult,
                                                   op1=mybir.AluOpType.mult,
                                                   accum_out=rs2[:, 0:1])
                    nc.vector.scalar_tensor_tensor(out=cys[:], in0=t1[:],
                                                   scalar=1.0, in1=dy[:],
                                                   op0=mybir.AluOpType.mult,
                                                   op1=mybir.AluOpType.mult,
                                                   accum_out=rs2[:, 1:2])
                else:
                    nc.vector.memset(rs2[:], 0.0)
                # one-hot row->segment, accumulate into acc
                nc.vector.tensor_scalar(out=onehot[:], in0=iota_f[:],
                                        scalar1=rm[:, 3:4], scalar2=None,
                                        op0=mybir.AluOpType.is_equal)
                nc.tensor.matmul(acc[:], rs2[:], onehot[:],
                                 start=(tau == 0), stop=(tau == T - 1),
                                 skip_group_check=True)

            acc_sb = cpool.tile([2, F], FP32)
            nc.vector.tensor_copy(out=acc_sb[:], in_=acc[:])
            nc.sync.dma_start(out=d_out[:], in_=acc_sb[:])

    nc.compile()
    return nc


def _host_prep(t, pos, poi_t, poi_pos, batch):
    """Shard + pad at graph boundaries. Returns per-core input dicts and T."""
    t = np.ascontiguousarray(np.asarray(t, dtype=np.float32))
    pos = np.ascontiguousarray(np.asarray(pos, dtype=np.float32))
    poi_t = np.asarray(poi_t, dtype=np.float32)
    poi_pos = np.asarray(poi_pos, dtype=np.float32)
    batch = np.asarray(batch)

    bounds = np.searchsorted(batch, np.arange(B + 1)).astype(np.int64)
    counts = np.diff(bounds)                       # [B]
    rows_per_seg = -(-counts // F)                 # ceil, 0 for empty segs

    core_rows = [int(rows_per_seg[k * SEGS:(k + 1) * SEGS].sum())
                 for k in range(NCORES)]
    R_needed = max(core_rows)
    T = -(-R_needed // P)
    R = T * P

    per_core = []
    for k in range(NCORES):
        s0, s1 = k * SEGS, (k + 1) * SEGS
        rs = rows_per_seg[s0:s1]
        nrows = int(rs.sum())
        seg_of_row = np.repeat(np.arange(s0, s1), rs)          # [nrows]
        row_in_seg = (np.arange(nrows)
                      - np.repeat(np.cumsum(rs) - rs, rs))     # 0,1,.. per seg
        row_node0 = bounds[seg_of_row] + row_in_seg * F

        pad = R - nrows
        seg_of_row = np.concatenate(
            [seg_of_row, np.full(pad, s1 - 1, np.int64)])
        row_node0 = np.concatenate([row_node0, np.full(pad, -1, np.int64)])

        nidx = row_node0[:, None] + np.arange(F)[None, :]       # [R, F]
        row_end = bounds[seg_of_row + 1]
        valid = (row_node0[:, None] >= 0) & (nidx < row_end[:, None])
        nidx_c = np.where(valid, nidx, 0)

        seg_pt = poi_t[seg_of_row]
        seg_px = poi_pos[seg_of_row, 0]
        seg_py = poi_pos[seg_of_row, 1]

        tt = np.where(valid, t[nidx_c], seg_pt[:, None]).astype(np.float32)
        px = np.where(valid, pos[nidx_c, 0], seg_px[:, None]).astype(np.float32)
        py = np.where(valid, pos[nidx_c, 1], seg_py[:, None]).astype(np.float32)
        rmeta = np.stack([-seg_pt, -seg_px, -seg_py,
                          (seg_of_row - s0).astype(np.float32)],
                         axis=1).astype(np.float32)
        per_core.append({"tt": tt, "px": px, "py": py, "rmeta": rmeta})
    return per_core, T


def _stationaries(W1, b1, W2, b2, W3, b3, W4, b4, W5, b5):
    W1, W2, W3, W4, W5 = [np.asarray(w, np.float32) for w in (W1, W2, W3, W4, W5)]
    b1, b2, b3, b4, b5 = [np.asarray(b, np.float32) for b in (b1, b2, b3, b4, b5)]
    s1d = np.zeros((128, 120 * 11), np.float32)
    s1r = np.zeros((128, 120 * 11), np.float32)
    for g, (g0, gs) in enumerate(GROUPS):
        for c in range(gs):
            s1d[g0 + c, 120 * g + 10 * c:120 * g + 10 * c + 10] = W1[:, 0]
            s1r[g0 + c, 120 * g + 10 * c:120 * g + 10 * c + 10] = W1[:, 1]
    s2 = np.zeros((60, 120), np.float32)
    for c in range(6):
        s2[10 * c:10 * c + 10, 20 * c:20 * c + 20] = W2.T
    s2b = np.zeros((120, 120), np.float32)
    s2b[60:120, :] = s2
    s3a = np.zeros((120, 124), np.float32)
    s3b = np.zeros((120, 124), np.float32)
    for c in range(6):
        s3a[20 * c:20 * c + 20, 10 * c:10 * c + 10] = W3.T
        s3b[20 * c:20 * c + 20, 64 + 10 * c:64 + 10 * c + 10] = W3.T
    s4 = np.zeros((124, 60), np.float32)
    for c in range(6):
        s4[10 * c:10 * c + 10, 5 * c:5 * c + 5] = W4.T
    for c in range(6):
        s4[64 + 10 * c:64 + 10 * c + 10, 5 * (6 + c):5 * (6 + c) + 5] = W4.T
    s5 = np.zeros((60, 128 * 11), np.float32)
    for g, (g0, gs) in enumerate(GROUPS):
        for c in range(gs):
            s5[5 * c:5 * c + 5, 128 * g + g0 + c] = W5[0]
    bias = np.zeros((128, 5), np.float32)
    bias[:120, 0] = np.tile(b1, 12)
    bias[:120, 1] = np.tile(b2, 6)
    bias[:60, 2] = np.tile(b3, 6)
    bias[64:124, 2] = np.tile(b3, 6)
    bias[:60, 3] = np.tile(b4, 12)
    bias[:, 4] = b5[0]
    return {"s1d": s1d, "s1r": s1r, "s2": s2, "s2b": s2b, "s3a": s3a,
            "s3b": s3b, "s4": s4, "s5": s5, "bias": bias}


_NC_CACHE = {}


def kernel(t, pos, poi_t, poi_pos, batch,
           W1, b1, W2, b2, W3, b3, W4, b4, W5, b5):
    per_core, T = _host_prep(t, pos, poi_t, poi_pos, batch)
    sta = _stationaries(W1, b1, W2, b2, W3, b3, W4, b4, W5, b5)

    if T not in _NC_CACHE:
        _NC_CACHE[T] = build_nc(T)
    nc = _NC_CACHE[T]

    in_maps = [{**core_inputs, **sta} for core_inputs in per_core]
    res = run_bass_kernel_spmd(nc, in_maps, list(range(NCORES)))
    global LAST_RESULT
    LAST_RESULT = res

    out = np.zeros((B, 2), np.float32)
    for k in range(NCORES):
        part = res.results[k]["part"]          # [2, 512]
        out[k * SEGS:(k + 1) * SEGS, :] = part.T
    return out
